# revision 14
# baseline (speedup 1.0000x reference)
"""DCNv2 (modulated deformable conv) forward on 8 Trainium2 NeuronCores.

Data-parallel over batch (B=8, one batch per core).  All GEMM work in fp8
DoubleRow (0.5 cycles/row vs bf16's 1.0):

  1. Phase 1: z_k[p, oc] = x^T W_k per tap with fp8 error feedback:
       psum = x_hi (x) w_hi  +  (16 x_lo) (x) (w_hi/16)  +  x_hi (x) w_lo
     (w pre-scaled by 256 so e4m3 stays normal; output rescaled on host).
     3 DoubleRow matmuls per 2-tap group (N = 512).  The f32 psum is split
     into z_hi = e4m3(psum) (Act) and z_lo = e5m2(psum - z_hi) (DVE).
  2. Phase 2: bilinear sample + mask + 9-tap reduce as windowed scatter-matrix
     matmuls over a 4-chunk (8 image row) window per tap.  Per (tap, tile):
       mm1: (S_hi[cc],  S_hi[cc+1]) x (z_hi[cc],  z_hi[cc+1])
       mm2: (S_hi[X],   S_hi[Y])    x (z_hi[X],   z_hi[Y])
       mm3: (S_lo[cc],  S_lo[cc+1]) x (z_hi[cc],  z_hi[cc+1])
       mm4: (S_hi[cc],  S_hi[cc+1]) x (z_lo[cc],  z_lo[cc+1])
     where (cc, cc+1) is the statically chosen max-energy adjacent chunk pair
     (~96% of sample-weight energy) getting full S_lo / z_lo error feedback.
     4 DoubleRow matmuls and only 6 streamed S planes per (tap, tile).
  3. Corners falling outside their 8-row window (~340/core), the bias, the
     2^-8 weight rescale and the output transpose are applied on the host.

Per 128-position tile: PE = 9 + 36 DoubleRow matmuls = 3.36 us vs the
bf16-phase-1 / 10-plane baseline's 5.28 us.
"""

from contextlib import ExitStack

import ml_dtypes
import numpy as np

import concourse.bass as bass
import concourse.bacc as bacc
import concourse.mybir as mybir
import concourse.tile as tile
from concourse.bass_utils import run_bass_kernel_spmd

F32 = mybir.dt.float32
FP8 = mybir.dt.float8e4
# e4m3 subnormal-dense operands fault the PE DoubleRow path (HW-bisected in a
# previous session); e5m2's normal range reaches 2^-14 so residues stay
# normal, and DoubleRow allows mixed e4m3 x e5m2 operands.
FP8L = mybir.dt.float8e5
PM = mybir.MatmulPerfMode.DoubleRow
NP_FP8 = ml_dtypes.float8_e4m3
NP_FP8L = ml_dtypes.float8_e5m2

# problem constants (hardcoded per harness contract)
B = 8
C = 256
OC = 256
H = W = 64
HW = H * W
K = 9
KH = KW = 3
PAD = 1

JT = 32            # 128-position output tiles (= z chunks)
CHW = 4            # window size in chunks (8 image rows)
NPL = 6            # S planes streamed per (tap, tile)
ZR = 7             # z ring depth in chunks
PIPE = 3           # phase2 runs PIPE tiles behind phase1 (needs phase1(jt+2))
WS = 64.0          # weight pre-scale (2^6): keeps e4m3(w) normal while
                   # |z'| stays well under e4m3's 240 max (peak ~75)
E4_MIN = 2.0 ** -6  # e4m3 min normal; flush below to keep operands subnormal-free
D_KY = (-2, -1, -1)  # window base: cb = clip(jt + D_KY[ky], 0, JT-CHW)

# phase-1 tap groups: psum tile per group (multi-bank); matmuls are issued
# per 2-tap half (N = 512 f32 = exactly one psum bank) but Act/DVE drain the
# whole group in one instruction each (Pool cannot access PSUM on TRN2).
P1_GROUPS = ((0, 4), (4, 4), (8, 1))

_program_cache = {}
_plan_cache = {}


def _chunk_mass(ky, d):
    """Static sample-weight-energy share of each window chunk for tap row ky
    when the window base is cb = jt + d.  Offsets ~ N(0,1); mask/x-direction
    factors are chunk-independent."""
    g = np.linspace(-6.0, 6.0, 2401)
    pdf = np.exp(-0.5 * g * g)
    pdf /= pdf.sum()
    m = np.zeros(CHW)
    for r in (0, 1):
        py = r + ky - 1 + g
        y0 = np.floor(py).astype(int)
        fy = py - y0
        for rowc, wgt in ((y0, 1 - fy), (y0 + 1, fy)):
            c = (rowc - 2 * d) // 2
            valid = (c >= 0) & (c < CHW)
            np.add.at(m, c[valid], (0.5 * pdf * wgt * wgt)[valid])
    return m


def _plan():
    """Per (ky, jt): (cb, cc, X, Y, slots).  cc = start of the max-energy
    adjacent chunk pair whose ring slots don't wrap (gets S_lo/z_lo
    feedback); X, Y = remaining chunks ordered by ascending ring slot so all
    rhs pair strides are positive."""
    if "plan" in _plan_cache:
        return _plan_cache["plan"]
    plan = {}
    for ky in range(KH):
        for jt in range(JT):
            cb = min(max(jt + D_KY[ky], 0), JT - CHW)
            slots = [(cb + c) % ZR for c in range(CHW)]
            m = _chunk_mass(ky, cb - jt)
            cands = [c for c in range(CHW - 1) if slots[c + 1] == slots[c] + 1]
            cc = max(cands, key=lambda c: m[c] + m[c + 1])
            rest = [c for c in range(CHW) if c not in (cc, cc + 1)]
            X, Y = sorted(rest, key=lambda c: slots[c])
            plan[(ky, jt)] = (cb, cc, X, Y, slots)
    _plan_cache["plan"] = plan
    return plan


def build_program():
    if "nc" in _program_cache:
        return _program_cache["nc"]
    plan = _plan()
    nc = bacc.Bacc("TRN2", target_bir_lowering=False, debug=False)

    # x: [part, jt, (hi, lo16), channel-half, j]
    x_d = nc.dram_tensor("x8", [128, JT, 2, 2, 128], FP8, kind="ExternalInput")
    whi_d = nc.dram_tensor("whi", [128, 2, K * OC], FP8, kind="ExternalInput")
    wh16_d = nc.dram_tensor("wh16", [128, 2, K * OC], FP8, kind="ExternalInput")
    wlo_d = nc.dram_tensor("wlo", [128, 2, K * OC], FP8L, kind="ExternalInput")
    s_d = nc.dram_tensor("s", [128, JT, K, NPL, 128], FP8, kind="ExternalInput")
    out_d = nc.dram_tensor("out", [JT, 128, OC], F32, kind="ExternalOutput")

    with tile.TileContext(nc) as tc, ExitStack() as ctx:
        sp = ctx.enter_context(tc.tile_pool(name="sbuf", bufs=1))
        x_p = ctx.enter_context(tc.tile_pool(name="xs", bufs=7))
        s_p = ctx.enter_context(tc.tile_pool(name="ss", bufs=PIPE + 2))
        ost_p = ctx.enter_context(tc.tile_pool(name="ost", bufs=4))
        psA_p = ctx.enter_context(tc.tile_pool(name="psA", bufs=3, space="PSUM"))
        psB_p = ctx.enter_context(tc.tile_pool(name="psB", bufs=1, space="PSUM"))
        ps2_p = ctx.enter_context(tc.tile_pool(name="ps2", bufs=1, space="PSUM"))

        whi_sb = sp.tile([128, 2, K * OC], FP8)
        wh16_sb = sp.tile([128, 2, K * OC], FP8)
        wlo_sb = sp.tile([128, 2, K * OC], FP8L)
        zhi = sp.tile([128, ZR, K, OC], FP8, name="zhi")
        zlo = sp.tile([128, ZR, K, OC], FP8L, name="zlo")

        s_tiles = {}
        x_tiles = {}

        def x_fetch(jt):
            xt = x_p.tile([128, 2, 2, 128], FP8, name=f"x_{jt}", tag="x")
            nc.sync.dma_start(xt[:], x_d.ap()[:, jt])
            x_tiles[jt] = xt

        def s_fetch(jt):
            st = s_p.tile([128, K, NPL, 128], FP8, name=f"s_{jt}", tag="s")
            nc.gpsimd.dma_start(st[:], s_d.ap()[:, jt])
            s_tiles[jt] = st

        def phase1(jt):
            slot = jt % ZR
            xt = x_tiles[jt]
            for k0, nk in P1_GROUPS:
                pool = psA_p if nk == 4 else psB_p
                ps = pool.tile([128, nk * OC], F32)
                # matmul outputs must stay within one 2KB psum bank: issue
                # per 2-tap (512-column) half
                for h0 in range(0, nk, 2):
                    nh = min(2, nk - h0)
                    pv = ps[:, h0 * OC:(h0 + nh) * OC]
                    o0, o1 = (k0 + h0) * OC, (k0 + h0 + nh) * OC
                    nc.tensor.matmul(pv, xt[:, 0], whi_sb[:, :, o0:o1],
                                     start=True, stop=False, perf_mode=PM)
                    nc.tensor.matmul(pv, xt[:, 1], wh16_sb[:, :, o0:o1],
                                     start=False, stop=False, perf_mode=PM)
                    nc.tensor.matmul(pv, xt[:, 0], wlo_sb[:, :, o0:o1],
                                     start=False, stop=True, perf_mode=PM)
                zh = zhi[:, slot, k0:k0 + nk, :]
                nc.scalar.copy(zh, ps[:])
                nc.vector.tensor_tensor(
                    out=zlo[:, slot, k0:k0 + nk, :],
                    in0=ps[:],
                    in1=zh,
                    op=mybir.AluOpType.subtract,
                )

        def phase2(jt):
            st = s_tiles.pop(jt)
            # last tile: split by oc so the final copy+DMA drain overlaps
            oparts = ((0, OC),) if jt < JT - 1 else ((0, 128), (128, OC))
            for oc0, oc1 in oparts:
                ps = ps2_p.tile([128, oc1 - oc0], F32)
                mms = []
                for k in range(K):
                    cb, cc, Xc, Yc, sl = plan[(k // KW, jt)]
                    pairs = ((0, zhi, cc), (2, zhi, None), (4, zhi, cc),
                             (0, zlo, cc))
                    for pl, zt, c0 in pairs:
                        if c0 is None:
                            sa, sb = sl[Xc], sl[Yc]
                        else:
                            sa, sb = sl[c0], sl[c0 + 1]
                        ap = zt[:]
                        rhs = bass.AP(ap.tensor, (sa * K + k) * OC + oc0,
                                      [[ZR * K * OC, 128],
                                       [(sb - sa) * K * OC, 2],
                                       [1, oc1 - oc0]])
                        mms.append((st[:, k, pl:pl + 2, :], rhs))
                for i, (lhsT, rhs) in enumerate(mms):
                    nc.tensor.matmul(ps[:], lhsT, rhs, start=(i == 0),
                                     stop=(i == len(mms) - 1), perf_mode=PM)
                ot = ost_p.tile([128, oc1 - oc0], F32)
                nc.scalar.copy(ot[:], ps[:])
                nc.sync.dma_start(out_d.ap()[jt, :, oc0:oc1], ot[:])

        XP = 6  # x prefetch depth
        for jt in range(XP):
            x_fetch(jt)
            if jt == 0:
                # stage w loads so phase1(0)'s first tap group unblocks early
                HALF = 4 * OC
                nc.sync.dma_start(whi_sb[:, :, :HALF], whi_d.ap()[:, :, :HALF])
                nc.gpsimd.dma_start(wh16_sb[:, :, :HALF],
                                    wh16_d.ap()[:, :, :HALF])
                nc.scalar.dma_start(wlo_sb[:, :, :HALF],
                                    wlo_d.ap()[:, :, :HALF])
                nc.sync.dma_start(whi_sb[:, :, HALF:], whi_d.ap()[:, :, HALF:])
                nc.gpsimd.dma_start(wh16_sb[:, :, HALF:],
                                    wh16_d.ap()[:, :, HALF:])
                nc.scalar.dma_start(wlo_sb[:, :, HALF:],
                                    wlo_d.ap()[:, :, HALF:])
        for jt in range(JT + PIPE):
            if jt < JT:
                if jt + XP < JT:
                    x_fetch(jt + XP)
                s_fetch(jt)
                phase1(jt)
                x_tiles.pop(jt)
            if jt >= PIPE:
                phase2(jt - PIPE)

    nc.compile()
    _program_cache["nc"] = nc
    return nc


def _sample_geometry(offset_b, mask_b):
    """Corner rows/cols/weights for every (corner, tap, position)."""
    off = offset_b.reshape(K, 2, H, W).astype(np.float64)
    m = mask_b.reshape(K, H, W).astype(np.float64)
    oy = np.arange(H, dtype=np.float64) - PAD
    ox = np.arange(W, dtype=np.float64) - PAD
    ky = np.repeat(np.arange(KH, dtype=np.float64), KW)
    kx = np.tile(np.arange(KW, dtype=np.float64), KH)
    py = ky[:, None, None] + oy[None, :, None] + off[:, 0]
    px = kx[:, None, None] + ox[None, None, :] + off[:, 1]
    y0 = np.floor(py).astype(np.int64)
    x0 = np.floor(px).astype(np.int64)
    wy = py - y0
    wx = px - x0
    ys, xs, ws = [], [], []
    for dy in (0, 1):
        for dx in (0, 1):
            yc = (y0 + dy).reshape(K, HW)
            xc = (x0 + dx).reshape(K, HW)
            w = ((wy if dy else 1 - wy) * (wx if dx else 1 - wx) * m).reshape(K, HW)
            valid = (yc >= 0) & (yc < H) & (xc >= 0) & (xc < W)
            ys.append(yc)
            xs.append(xc)
            ws.append(np.where(valid, w, 0.0))
    return np.stack(ys), np.stack(xs), np.stack(ws)  # [4, K, HW]


def _static_tables():
    """CB[k, jt] window base and PLANE[k, jt, chunk] chunk->plane map."""
    if "tables" in _plan_cache:
        return _plan_cache["tables"]
    plan = _plan()
    CB = np.zeros((K, JT), dtype=np.int64)
    PL = np.zeros((K, JT, CHW), dtype=np.int64)
    for k in range(K):
        for jt in range(JT):
            cb, cc, Xc, Yc, _ = plan[(k // KW, jt)]
            CB[k, jt] = cb
            PL[k, jt, cc] = 0
            PL[k, jt, cc + 1] = 1
            PL[k, jt, Xc] = 2
            PL[k, jt, Yc] = 3
    _plan_cache["tables"] = (CB, PL)
    return CB, PL


def _build_S(offset_b, mask_b):
    """S[zp_part, jt, k, plane, j] fp8 + dropped out-of-window corners."""
    CB, PL = _static_tables()
    yc, xc, w = _sample_geometry(offset_b, mask_b)  # [4, K, HW]
    p = np.arange(HW)
    jt = p // 128
    j = p % 128
    cb = CB[:, jt]                   # [K, HW]
    zrow = yc - 2 * cb[None]         # window-relative row
    zp = zrow * W + xc               # [4, K, HW] in [0, CHW*128)
    inwin = (zrow >= 0) & (zrow < 2 * CHW)
    ok = (w != 0) & inwin
    drop = (w != 0) & ~inwin

    kk = np.arange(K)[None, :, None]
    zp_s = np.where(ok, zp, 0)
    chunk = zp_s >> 7
    zpin = zp_s & 127
    plane = PL[kk, jt[None, None], chunk]
    idx = (((zpin * JT + jt[None, None]) * K + kk) * NPL + plane) * 128 \
        + j[None, None]
    S = np.bincount(idx[ok].ravel(), weights=w[ok].ravel(),
                    minlength=128 * JT * K * NPL * 128)
    S = S.reshape(128, JT, K, NPL, 128).astype(np.float32)
    S_dev = S.astype(NP_FP8)
    # planes 4:6 = S_lo of the feedback chunk pair (= planes 0:2)
    S_dev[:, :, :, 4:6] = (S[:, :, :, 0:2]
                           - S_dev[:, :, :, 0:2].astype(np.float32)).astype(NP_FP8)

    ci, ki, pi = np.nonzero(drop)
    dropped = (ki, pi, yc[ci, ki, pi], xc[ci, ki, pi], w[ci, ki, pi])
    return S_dev, dropped


def _flushed_e4(a):
    q = a.astype(NP_FP8)
    q[np.abs(a) < E4_MIN] = 0
    return q


def _prep_core_inputs(x_b, offset_b, mask_b, weight):
    xf = x_b.reshape(C, HW).astype(np.float32)
    xhi = _flushed_e4(xf)
    xlo16 = _flushed_e4(16.0 * (xf - xhi.astype(np.float32)))
    # [c, pos] -> [part, jt, (hi,lo), half, j]
    def dev_x(a):
        return a.reshape(2, 128, JT, 128).transpose(1, 2, 0, 3)
    x8 = np.ascontiguousarray(
        np.stack([dev_x(xhi), dev_x(xlo16)], axis=2))

    wkc = (weight.reshape(OC, C, K).astype(np.float32) * WS).transpose(1, 2, 0)
    whi = _flushed_e4(wkc)                      # [C, K, OC]
    whi_f = whi.astype(np.float32)
    wh16 = (whi_f / 16.0).astype(NP_FP8)
    wlo = (wkc - whi_f).astype(NP_FP8L)

    def dev_w(a):
        return np.ascontiguousarray(
            a.reshape(2, 128, K * OC).transpose(1, 0, 2))

    S_dev, dropped = _build_S(offset_b, mask_b)
    return {"x8": x8, "whi": dev_w(whi), "wh16": dev_w(wh16),
            "wlo": dev_w(wlo), "s": S_dev}, dropped


def _host_fixup(out_b, x_b, weight, dropped):
    """Add the exact contribution of corners outside their window."""
    ki, pi, yi, xi, wi = dropped
    if len(ki) == 0:
        return
    wk = weight.reshape(OC, C, K)
    q = (yi * W + xi).astype(np.int64)
    for k in np.unique(ki):
        m = ki == k
        cols = x_b[:, q[m]]                      # (C, n)
        contrib = (wk[:, :, k] @ cols) * wi[m][None]  # (OC, n)
        np.add.at(out_b.T, pi[m], contrib.T.astype(np.float32))


def kernel(x, offset, mask, weight, bias):
    x = np.asarray(x, dtype=np.float32)
    offset = np.asarray(offset, dtype=np.float32)
    mask = np.asarray(mask, dtype=np.float32)
    weight = np.asarray(weight, dtype=np.float32)
    bias = np.asarray(bias, dtype=np.float32)

    nc = build_program()
    in_maps, droppeds = [], []
    for b in range(B):
        m, dropped = _prep_core_inputs(x[b], offset[b], mask[b], weight)
        in_maps.append(m)
        droppeds.append(dropped)
    res = run_bass_kernel_spmd(nc, in_maps, core_ids=list(range(B)))

    out = np.empty((B, OC, HW), dtype=np.float32)
    for b in range(B):
        o = res.results[b]["out"]  # (JT, 128, OC) j-major
        out[b] = np.ascontiguousarray(o.reshape(HW, OC).T) * (1.0 / WS)
        _host_fixup(out[b], x[b].reshape(C, HW), weight, droppeds[b])
    out += bias[None, :, None]
    return out.reshape(B, OC, H, W)
